# revision 7
# baseline (speedup 1.0000x reference)
"""Causal self-attention with relative position bias on 8 Trainium2 cores.

Sharding: batch B=4 x head-group (2 groups of 8 heads) -> 8 cores.
Each core: QKV projection for its (batch, head-group), attention for its 8
heads, pairwise AllGather of per-head outputs, then the output projection for
its 512 output channels over all 1024 tokens.  Host concatenates channel
halves per batch.

Key tricks:
- All big matmuls run as float32r (full-rate fp32 on the PE array).
- Scores are computed transposed (keys on partitions) so softmax sums and the
  PV matmul need no transposes: the denominator comes from a ones-column
  appended to V, the bias+causal mask is preloaded into PSUM via an
  identity-matmul from a Toeplitz-shifted DMA view of a per-head table, and
  queries are read in reverse (negative stride) so every DMA partition step
  stays positive.
"""

import numpy as np

import concourse.bass as bass
import concourse.bacc as bacc
import concourse.tile as tile
from concourse import mybir
from concourse.bass_utils import run_bass_kernel_spmd
from concourse.masks import make_identity

F32 = mybir.dt.float32
F32R = mybir.dt.float32r

B, T, C = 4, 1024, 1024
H = 16
D = 64
HPC = 8          # heads per core
NEG = -8192.0    # causal mask fill (exp(0.125 * (s + NEG)) == 0 in fp32)

IB_N = 2         # i-blocks of 512 queries
CT_N = 8         # contraction tiles of 128 channels
TT_N = 8         # token tiles of 128

STAGED_LEN = 2047
DGM_W = 1408     # max slice base (896) + 512


def _rev_last(ap):
    """AP reading `ap` with its innermost dim reversed (negative stride)."""
    dims = [list(d) for d in ap.ap]
    fstep, fcount = dims[-1]
    dims[-1] = [-fstep, fcount]
    return bass.AP(
        tensor=ap.tensor,
        offset=ap.offset + fstep * (fcount - 1),
        ap=dims,
    )


def _shifted_window(dram_ap, elem_offset, rows, cols):
    """AP over flat DRAM: out[p, m] = dram[elem_offset + p + m] (overlapping)."""
    return bass.AP(
        tensor=dram_ap.tensor,
        offset=dram_ap.offset + elem_offset,
        ap=[[1, rows], [1, cols]],
    )


def build(nc: bass.Bass):
    x = nc.dram_tensor("x", [T, C], F32, kind="ExternalInput")
    wt = nc.dram_tensor("wt", [C, 1536], F32, kind="ExternalInput")
    bqkv = nc.dram_tensor("bqkv", [1536], F32, kind="ExternalInput")
    pwt = nc.dram_tensor("pwt", [C, 512], F32, kind="ExternalInput")
    pb = nc.dram_tensor("pb", [512], F32, kind="ExternalInput")
    rel = nc.dram_tensor("rel", [1152, 512], F32, kind="ExternalInput")
    y = nc.dram_tensor("y", [T, 512], F32, kind="ExternalOutput")

    with tile.TileContext(nc) as tc:
        with tc.tile_pool(name="consts", bufs=1) as consts, \
             tc.tile_pool(name="big", bufs=1) as big, \
             tc.tile_pool(name="dram", bufs=1, space="DRAM") as dram:
            # ---- constants
            ident_f = consts.tile([128, 128], F32)
            make_identity(nc, ident_f)
            ident_r = consts.tile([128, 128], F32R)
            nc.scalar.copy(ident_r[:], ident_f[:])
            ones_f = consts.tile([1, 128], F32)
            nc.vector.memset(ones_f, 1.0)
            ones_r = consts.tile([1, 128], F32R)
            nc.scalar.copy(ones_r[:], ones_f[:])
            antid_f = consts.tile([128, 128], F32)
            nc.gpsimd.memset(antid_f, 0.0)
            nc.gpsimd.affine_select(
                out=antid_f, in_=antid_f,
                compare_op=mybir.AluOpType.not_equal,
                fill=1.0, base=-127,
                pattern=[[1, 128]], channel_multiplier=1,
            )

            # ---- persistent big buffers
            qt_sb = big.tile([128, 4, T], F32R)       # [d within head pair, hp, t]
            kt_sb = big.tile([128, 4, T], F32R)
            v_sb = big.tile([128, TT_N, HPC, 65], F32R)  # V + ones col
            ot_sb = big.tile([128, 4, T], F32R)       # attention out^T (natural t)

            og_dram = dram.tile([512, T], F32R)       # my head-group O^T
            otf_dram = dram.tile([1024, T], F32R)     # gathered full O^T
            staged_dram = dram.tile([HPC, STAGED_LEN], F32R)

            with tc.tile_pool(name="xt", bufs=1) as xt_pool, \
                 tc.tile_pool(name="wtp", bufs=1) as wt_pool:
                xt_sb = xt_pool.tile([128, CT_N, T], F32R)  # [c within ct, ct, t]
                wt_sb = wt_pool.tile([128, CT_N, 1536], F32R)
                for i3 in range(3):
                    nc.gpsimd.dma_start(
                        out=wt_sb[:, :, 512 * i3 : 512 * i3 + 512],
                        in_=wt[:, 512 * i3 : 512 * i3 + 512].rearrange(
                            "(ct p) n -> p ct n", p=128
                        ),
                    )

                # =====================================================
                # Phase 0a: transpose x into xT (c on partitions)
                # =====================================================
                with tc.tile_pool(name="ph0", bufs=2) as ph0, \
                     tc.tile_pool(name="ph0ps", bufs=4, space="PSUM") as ph0ps:
                    for tt in range(TT_N):
                        x_row = ph0.tile([128, C], F32R, tag="xrow")
                        nc.gpsimd.dma_start(
                            out=x_row, in_=x[128 * tt : 128 * tt + 128, :]
                        )
                        for cq in range(2):
                            pst = ph0ps.tile([128, 512], F32R, tag="xposeps")
                            for q in range(4):
                                ct = 4 * cq + q
                                nc.tensor.transpose(
                                    pst[:, 128 * q : 128 * q + 128],
                                    x_row[:, 128 * ct : 128 * ct + 128],
                                    ident_r[:],
                                )
                            nc.scalar.copy(
                                xt_sb[:, 4 * cq : 4 * cq + 4,
                                      128 * tt : 128 * tt + 128],
                                pst[:].rearrange("p (q j) -> p q j", q=4),
                            )

                # =====================================================
                # Phase 0b: staged_rev table from rel_pos_emb
                # staged[w] = 8*sum_d rel[w, 64h+d] for w>=1023, NEG below;
                # staged_rev[k] = staged[2046-k]
                # =====================================================
                with tc.tile_pool(name="tbl", bufs=2) as tbl, \
                     tc.tile_pool(name="tblps", bufs=2, space="PSUM") as tblps:
                    staged_f = tbl.tile([HPC, STAGED_LEN], F32, tag="staged")
                    for wtp in range(9):  # w = 896 + 128*wtp + p
                        rtile = tbl.tile([128, 512], F32, tag="rel")
                        nc.sync.dma_start(
                            out=rtile, in_=rel[128 * wtp : 128 * wtp + 128, :]
                        )
                        red = tbl.tile([128, HPC], F32, tag="red")
                        nc.vector.reduce_sum(
                            out=red[:],
                            in_=rtile[:].rearrange("p (h d) -> p h d", h=HPC),
                            axis=mybir.AxisListType.X,
                        )
                        # pst[h, j] = red[127-j, h]  (w descending within chunk)
                        pst = tblps.tile([HPC, 128], F32, tag="tblps")
                        nc.tensor.transpose(pst[:], red[:, 0:HPC], antid_f[:])
                        kbase = 1023 - 128 * wtp
                        j0 = 0 if kbase >= 0 else -kbase
                        nc.scalar.mul(
                            staged_f[:, kbase + j0 : kbase + 128],
                            pst[:, j0:128],
                            8.0,
                        )
                    nc.vector.memset(staged_f[:, 1024:STAGED_LEN], NEG)
                    staged_r = tbl.tile([HPC, STAGED_LEN], F32R, tag="stagedr")
                    nc.scalar.copy(staged_r[:], staged_f[:])
                    nc.sync.dma_start(out=staged_dram[:], in_=staged_r[:])

                # =====================================================
                # Phase 1: QKV projections
                # =====================================================
                with tc.tile_pool(name="qkps", bufs=4, space="PSUM") as qkps, \
                     tc.tile_pool(name="onesps", bufs=1, space="PSUM") as onesps, \
                     tc.tile_pool(name="bia", bufs=1) as bia:
                    bq_sb = bia.tile([128, 4], F32)
                    bk_sb = bia.tile([128, 4], F32)
                    nc.sync.dma_start(
                        out=bq_sb,
                        in_=bqkv[0:512].rearrange("(hp p) -> p hp", p=128),
                    )
                    nc.sync.dma_start(
                        out=bk_sb,
                        in_=bqkv[512:1024].rearrange("(hp p) -> p hp", p=128),
                    )
                    bv_f = bia.tile([1, 512], F32)
                    nc.sync.dma_start(
                        out=bv_f, in_=bqkv[1024:1536].rearrange("(a n) -> a n", a=1)
                    )
                    bv_row = bia.tile([1, 512], F32R)
                    nc.vector.tensor_copy(out=bv_row[:], in_=bv_f[:])

                    for hp in range(4):
                        for tb in range(2):
                            for dst, wofs, bias_t in (
                                (qt_sb, 0, bq_sb),
                                (kt_sb, 512, bk_sb),
                            ):
                                ps = qkps.tile([128, 512], F32, tag="qk")
                                for ct in range(CT_N):
                                    nc.tensor.matmul(
                                        ps[:],
                                        wt_sb[:, ct,
                                              wofs + 128 * hp : wofs + 128 * hp + 128],
                                        xt_sb[:, ct, 512 * tb : 512 * tb + 512],
                                        start=(ct == 0),
                                        stop=(ct == CT_N - 1),
                                    )
                                nc.scalar.activation(
                                    dst[:, hp, 512 * tb : 512 * tb + 512],
                                    ps[:],
                                    mybir.ActivationFunctionType.Identity,
                                    bias=bias_t[:, hp : hp + 1],
                                )

                    # all-ones [128, HPC] for V's denominator column
                    ps1 = onesps.tile([128, HPC], F32, tag="ones")
                    nc.tensor.matmul(
                        ps1[:], ones_r[:, 0:128], ones_r[:, 0:HPC],
                        start=True, stop=True,
                    )
                    for tt in range(TT_N):
                        ps = qkps.tile([128, 512], F32, tag="qk")
                        for ct in range(CT_N):
                            nc.tensor.matmul(
                                ps[:],
                                xt_sb[:, ct, 128 * tt : 128 * tt + 128],
                                wt_sb[:, ct, 1024:1536],
                                start=(ct == 0),
                                stop=False,
                            )
                        nc.tensor.matmul(
                            ps[:], ones_r[:, 0:128], bv_row[:],
                            start=False, stop=True,
                        )
                        nc.scalar.copy(
                            v_sb[:, tt, :, 0:64],
                            ps[:].rearrange("p (h d) -> p h d", h=HPC),
                        )
                        nc.scalar.copy(v_sb[:, tt, :, 64], ps1[:])

            # =========================================================
            # Phase 2: attention per head
            # =========================================================
            with tc.tile_pool(name="at", bufs=2) as at, \
                 tc.tile_pool(name="ep", bufs=6) as ep, \
                 tc.tile_pool(name="sps", bufs=3, space="PSUM") as sps, \
                 tc.tile_pool(name="ops", bufs=2, space="PSUM") as ops, \
                 tc.tile_pool(name="nrm", bufs=4) as nrm:
                sdram_ap = staged_dram[:]
                for h in range(HPC):
                    hp, hl = h // 2, 64 * (h % 2)
                    dgm = at.tile([128, DGM_W], F32R, tag="dgm")
                    nc.sync.dma_start(
                        out=dgm,
                        in_=_shifted_window(sdram_ap, h * STAGED_LEN, 128, DGM_W),
                    )
                    for ib in range(IB_N):
                        jts = range(4) if ib == 0 else range(8)
                        po = ops.tile([65, 512], F32, tag="po")
                        for idx, jt in enumerate(jts):
                            ps = sps.tile([128, 512], F32, tag="s")
                            dbase = 512 - 512 * ib + 128 * jt
                            nc.tensor.matmul(
                                ps[:], ident_r[:],
                                dgm[:, dbase : dbase + 512],
                                start=True, stop=False,
                            )
                            nc.tensor.matmul(
                                ps[:],
                                kt_sb[hl : hl + 64, hp, 128 * jt : 128 * jt + 128],
                                _rev_last(
                                    qt_sb[hl : hl + 64, hp,
                                          512 * ib : 512 * ib + 512]
                                ),
                                start=False, stop=True,
                            )
                            e_t = ep.tile([128, 512], F32R, tag="e")
                            nc.scalar.activation(
                                e_t[:], ps[:],
                                mybir.ActivationFunctionType.Exp,
                                scale=0.125,
                            )
                            nc.tensor.matmul(
                                po[:],
                                v_sb[:, jt, h, :],
                                e_t[:],
                                start=(idx == 0),
                                stop=(idx == len(jts) - 1),
                                skip_group_check=True,
                            )
                        # normalize rows 0..63 by row 64 (reversed query order)
                        r_f = nrm.tile([1, 512], F32, tag="rf")
                        nc.vector.reciprocal(out=r_f[:], in_=po[64:65, :])
                        bc_sb = nrm.tile([64, 512], F32, tag="bc")
                        nc.gpsimd.partition_broadcast(bc_sb[:], r_f[:])
                        # un-reverse on write
                        nc.vector.tensor_mul(
                            _rev_last(
                                ot_sb[hl : hl + 64, hp, 512 * ib : 512 * ib + 512]
                            ),
                            po[0:64, :],
                            bc_sb[:],
                        )
                for hp in range(4):
                    nc.sync.dma_start(
                        out=og_dram[128 * hp : 128 * hp + 128, :],
                        in_=ot_sb[:, hp, :],
                    )

            # =========================================================
            # Phase 3: AllGather + output projection
            # =========================================================
            nc.gpsimd.collective_compute(
                "AllGather",
                mybir.AluOpType.bypass,
                replica_groups=[[0, 1], [2, 3], [4, 5], [6, 7]],
                ins=[og_dram.opt()],
                outs=[otf_dram.opt()],
            )
            with tc.tile_pool(name="pj", bufs=2) as pj, \
                 tc.tile_pool(name="otf", bufs=1) as otf_pool, \
                 tc.tile_pool(name="pjps", bufs=4, space="PSUM") as pjps:
                otf_sb = otf_pool.tile([128, CT_N, T], F32R)
                for ct in range(CT_N):
                    nc.sync.dma_start(
                        out=otf_sb[:, ct, :],
                        in_=otf_dram[128 * ct : 128 * ct + 128, :],
                    )
                pwt_sb = otf_pool.tile([128, CT_N, 512], F32R)
                nc.gpsimd.dma_start(
                    out=pwt_sb, in_=pwt[:, :].rearrange("(ct p) n -> p ct n", p=128)
                )
                pb_f = otf_pool.tile([1, 512], F32)
                nc.sync.dma_start(
                    out=pb_f, in_=pb[:].rearrange("(a n) -> a n", a=1)
                )
                pb_r = otf_pool.tile([1, 512], F32R)
                nc.vector.tensor_copy(out=pb_r[:], in_=pb_f[:])

                for tt in range(TT_N):
                    ps = pjps.tile([128, 512], F32, tag="y")
                    for ct in range(CT_N):
                        nc.tensor.matmul(
                            ps[:],
                            otf_sb[:, ct, 128 * tt : 128 * tt + 128],
                            pwt_sb[:, ct, :],
                            start=(ct == 0),
                            stop=False,
                        )
                    nc.tensor.matmul(
                        ps[:], ones_r[:, 0:128], pb_r[:], start=False, stop=True
                    )
                    yt = pj.tile([128, 512], F32, tag="yt")
                    nc.scalar.copy(yt[:], ps[:])
                    nc.sync.dma_start(
                        out=y[128 * tt : 128 * tt + 128, :], in_=yt
                    )
    return nc


def _host_inputs(x, qkv_w, qkv_b, proj_w, proj_b, rel_pos_emb):
    """Slice/relayout full inputs into per-core input maps (host side)."""
    wt_full = np.ascontiguousarray(qkv_w.T)       # (C, 3C)
    pwt_full = np.ascontiguousarray(proj_w.T)     # (C, C)
    in_maps = []
    for c in range(8):
        b, g = c // 2, c % 2
        cols = np.r_[
            512 * g : 512 * g + 512,
            1024 + 512 * g : 1024 + 512 * g + 512,
            2048 + 512 * g : 2048 + 512 * g + 512,
        ]
        rel_part = np.zeros((1152, 512), np.float32)
        rel_part[:1151] = rel_pos_emb[896:, 512 * g : 512 * g + 512]
        in_maps.append({
            "x": np.ascontiguousarray(x[b]),
            "wt": np.ascontiguousarray(wt_full[:, cols]),
            "bqkv": np.ascontiguousarray(qkv_b[cols]),
            "pwt": np.ascontiguousarray(pwt_full[:, 512 * g : 512 * g + 512]),
            "pb": np.ascontiguousarray(proj_b[512 * g : 512 * g + 512]),
            "rel": rel_part,
        })
    return in_maps


_NC_CACHE = []


def kernel(x, qkv_w, qkv_b, proj_w, proj_b, rel_pos_emb, _trace=False):
    x = np.asarray(x, np.float32)
    qkv_w = np.asarray(qkv_w, np.float32)
    qkv_b = np.asarray(qkv_b, np.float32)
    proj_w = np.asarray(proj_w, np.float32)
    proj_b = np.asarray(proj_b, np.float32)
    rel_pos_emb = np.asarray(rel_pos_emb, np.float32)

    if not _NC_CACHE:
        nc = bacc.Bacc("TRN2", target_bir_lowering=False, debug=False)
        build(nc)
        nc.finalize()
        _NC_CACHE.append(nc)
    nc = _NC_CACHE[0]

    in_maps = _host_inputs(x, qkv_w, qkv_b, proj_w, proj_b, rel_pos_emb)
    res = run_bass_kernel_spmd(
        nc, in_maps, core_ids=list(range(8)), trace=_trace
    )
    out = np.empty((B, T, C), np.float32)
    for b in range(B):
        out[b, :, 0:512] = res.results[2 * b]["y"]
        out[b, :, 512:1024] = res.results[2 * b + 1]["y"]
    if _trace:
        kernel._last_results = res
    return out
